# revision 18
# baseline (speedup 1.0000x reference)
"""Trainium2 Bass kernel for DGLFeatureGAT (dense GATv2 over complete graph).

Reference computation (per batch b, head h; N=64 nodes, D=128 feat dim):
    el = xn @ Wl,  er = xn @ Wr                      # [N, H, D]
    e[h,i,j] = sum_d a[h,d] * lrelu(el[j,h,d] + er[i,h,d])
    alpha = softmax_j(e);  rst[i,h,d] = sum_j alpha[h,i,j] el[j,h,d] + bias
    out = mean_h(rst) transposed to [D, N]

Exact decomposition (slope s=0.2):
    lrelu(z) = (1-s)*relu(z) + s*z, so with z = el_j + er_i:
    e = (1-s)*sum_d a_d relu(z_d) + s*u_j + s*v_i    (u = a.el, v = a.er)
      - v_i constant over j -> dropped (softmax invariant)
      - u_j enters as g_j = exp(s*u_j) multiplied into the aggregation rhs,
        with one extra rhs column accumulating the softmax normalizer.

Pair-space layout per (b, h) "unit" (4096 pairs = 8 chunks x 512):
    pair p = 512c + 64*lam + i.  Node-j slots use the bit-rotated index
    r = 32*jl0 + 4c + jhi (lam = 2*jhi + jl0): the e-reduce then lands e
    DENSE in PSUM ([128, 32]: partition 64*jl0+i, col 4c+jhi), one exp per
    unit, PE-transpose + copy + two standard-slice DMAs give pT2[r, 64u+i].
    All j-side tensors (el, elg, g, selector rows) consistently use
    r-indexing; i-side and the output stay in natural order.

Engines:
    z+relu per chunk, route table CH:
      A = PE z-form matmul (bf16 stationary [el;er], fp8 0/1 selector
          moving) + ACT Relu from PSUM (612ns)
      D = DVE: tensor_tensor add of broadcast views (elT slot-expanded +
          erT tiled, 2x) + tensor_scalar relu (4x), SBUF bf16 (491ns)
      P = Pool tensor_scalar relu(erT + elT32_col) per slot (8x184ns)
    e-reduce: 4 matmuls/chunk, STATIONARY = zabs [128,128] slice, moving =
    0.8*a_h column (out free size 1 -> ~free on PE).
    exp: one ACT call per unit [128, 32].  Aggregation fp32 on PE.

Sharding: pure data-parallel, B=32 -> 4 batches per core x 8 cores.
"""

import numpy as np
from contextlib import ExitStack

import concourse.bass as bass
import concourse.bacc as bacc
import concourse.tile as tile
from concourse import mybir
from concourse.bass_utils import run_bass_kernel_spmd

f32 = mybir.dt.float32
bf16 = mybir.dt.bfloat16
fp8 = mybir.dt.float8e4
Act = mybir.ActivationFunctionType

B, W, F, H, D = 32, 128, 64, 2, 128
NEG_SLOPE = 0.2
N_CORES = 8
B_LOC = B // N_CORES            # 4 batches per core
N = F                           # 64 nodes
NCHUNK = 8                      # 512-col chunks of the 4096 pair space
NU = B_LOC * H                  # 8 units per core

# blobA column layout (float32 bits; bf16 regions are packed); the fp8
# selector ships separately as blobS so head compute starts immediately.
OFF_X = 0                            # [128, 128]: x[b] bf16 [w, r] natural
OFF_XT = OFF_X + B_LOC * N // 2      # [128, 128]: x[b] bf16 slot-order tau
OFF_WLR = OFF_XT + B_LOC * N // 2    # [128, 257]: Wl | Wr | wl_u bf16
OFF_ACOL = OFF_WLR + 257             # [128, 1]: 0.8*a bf16 pair (h0, h1)
OFF_BIAS = OFF_ACOL + 1              # [128, 1]: fused output bias f32
OFF_I128 = OFF_BIAS + 1              # [128, 128]: identity f32
NCOLS = OFF_I128 + 128
NCOLS_S = N * N // 4                 # selector fp8-packed

# per-chunk z route (see module docstring).  P-chunks must be spread so
# Pool stays fed in program order.
CH = {
    0: ["A", "A", "A", "A", "A", "A", "A", "A"],
    1: ["A", "A", "P", "A", "A", "A", "P", "A"],
    2: ["A", "A", "A", "P", "A", "A", "A", "P"],
    3: ["P", "D", "A", "D", "P", "D", "A", "D"],
    4: ["A", "D", "P", "D", "A", "P", "D", "D"],
    5: ["D", "A", "D", "P", "D", "A", "P", "D"],
    6: ["P", "D", "D", "A", "P", "D", "A", "D"],
    7: ["D", "P", "D", "D", "D", "P", "D", "D"],
}

_cache = {}


def _build():
    if "nc" in _cache:
        return _cache["nc"]
    nc = bacc.Bacc("TRN2", target_bir_lowering=False, debug=False)
    blob_d = nc.declare_dram_parameter("blob", [128, NCOLS], f32,
                                       isOutput=False).ap()
    blobs_d = nc.declare_dram_parameter("blobS", [128, NCOLS_S], f32,
                                        isOutput=False).ap()
    y_d = nc.declare_dram_parameter("y", [B_LOC, D, F], f32,
                                    isOutput=True).ap()

    with tile.TileContext(nc) as tc, ExitStack() as ctx:
        sb1 = ctx.enter_context(tc.tile_pool(name="sb1", bufs=1))
        sbE = ctx.enter_context(tc.tile_pool(name="sbE", bufs=2 * B_LOC))
        sbZ = ctx.enter_context(tc.tile_pool(name="sbZ", bufs=2))
        sbU = ctx.enter_context(tc.tile_pool(name="sbU", bufs=B_LOC))
        psP = ctx.enter_context(tc.tile_pool(name="psP", bufs=2, space="PSUM"))
        psT = ctx.enter_context(tc.tile_pool(name="psT", bufs=2, space="PSUM"))
        psZ = ctx.enter_context(tc.tile_pool(name="psZ", bufs=3, space="PSUM"))
        psR = ctx.enter_context(tc.tile_pool(name="psR", bufs=1, space="PSUM"))

        blob = sb1.tile([128, NCOLS], f32, tag="blob")
        nc.sync.dma_start(blob[:], blob_d)
        blobS = sb1.tile([128, NCOLS_S], f32, tag="blobS")
        nc.sync.dma_start(blobS[:], blobs_d)

        def bl(off, w):
            return blob[:, off:off + w]

        xall = bl(OFF_X, B_LOC * N // 2).bitcast(bf16)       # [128, 256]
        xtau = bl(OFF_XT, B_LOC * N // 2).bitcast(bf16)      # [128, 256]
        wlr = bl(OFF_WLR, 257).bitcast(bf16)                 # [128, 514]
        ssel = blobS[:].bitcast(fp8)                         # [128, 4096]
        acol = bl(OFF_ACOL, 1).bitcast(bf16)                 # [128, 2]
        ident = bl(OFF_I128, 128)
        bias_ap = bl(OFF_BIAS, 1)

        pT2 = sb1.tile([N, NU * N], bf16, tag="pT2")
        y_all = sb1.tile([D, B_LOC * N], f32, tag="yall")
        Rbank = psR.tile([128, 64], f32, tag="r2")   # col 32*(u%2) + 4c + m

        elgs = {}
        elers = {}
        bcs = {}
        for b in range(B_LOC):
            has_a = {h: any(r == "A" for r in CH[2 * b + h]) for h in range(H)}
            has_d = {h: any(r == "D" for r in CH[2 * b + h]) for h in range(H)}
            has_p = {h: any(r == "P" for r in CH[2 * b + h]) for h in range(H)}

            xb = xall[:, b * N:(b + 1) * N]                  # [128, 64] bf16
            xbt = xtau[:, b * N:(b + 1) * N]                 # [128, 64] bf16

            proj = psP.tile([N, 512], f32, tag="sm", name="proj")
            nc.tensor.matmul(proj[:], xb, wlr[:, 0:512],
                             start=True, stop=True)
            proju = psP.tile([N, 2], f32, tag="sm", name="proju")
            nc.tensor.matmul(proju[:], xb, wlr[:, 512:514],
                             start=True, stop=True)

            # stacked [el; er] (rows 0:64 el, 64:128 er) bf16 for the A
            # route z-form; el half doubles as the 4x elg source.
            eler = sbE.tile([128, H * D], bf16, tag="eler")
            nc.vector.tensor_copy(eler[0:N, :], proj[:, 0:H * D])
            if any(has_a.values()):
                nc.vector.tensor_copy(eler[N:128, :],
                                      proj[:, H * D:2 * H * D])

            g_b = sbU.tile([N, H], f32, tag="g")   # g[r,h] = exp(s*u)
            nc.scalar.activation(g_b[:], proju[:], Act.Exp)

            # elg[r, 0:D] = el[r, :] * g_r ; elg[r, D] = g_r   (4x DVE)
            for h in range(H):
                elg = sbU.tile([N, D + 1], bf16, tag=f"elg{b}{h}",
                               name=f"elg{b}{h}", bufs=1)
                nc.vector.tensor_scalar(
                    elg[:, 0:D], eler[0:N, h * D:(h + 1) * D],
                    g_b[:, h:h + 1], None, mybir.AluOpType.mult)
                nc.vector.tensor_copy(elg[:, D:D + 1], g_b[:, h:h + 1])
                elgs[(b, h)] = elg

            # transposed projections (slot order tau) for the SBUF routes
            bc = {}
            for h in range(H):
                if not (has_d[h] or has_p[h]):
                    continue
                prT = psP.tile([128, 128], f32, tag="sm", name=f"prT{b}{h}")
                nc.tensor.matmul(prT[:, 0:N], wlr[:, h * D:(h + 1) * D],
                                 xbt, start=True, stop=True)
                nc.tensor.matmul(prT[:, N:2 * N],
                                 wlr[:, H * D + h * D:H * D + (h + 1) * D],
                                 xb, start=True, stop=True)
                erT = sbE.tile([128, N], bf16, tag="erT", name=f"erT{b}{h}")
                nc.vector.tensor_copy(erT[:], prT[:, N:2 * N])
                elT32 = sbE.tile([128, N], f32, tag="elT32",
                                 name=f"elT32{b}{h}")
                nc.vector.tensor_copy(elT32[:], prT[:, 0:N])
                bc[h] = (erT, elT32)
            elers[b] = eler
            bcs[b] = bc

        for b in range(B_LOC):
            eler = elers[b]
            bc = bcs[b]
            for h in range(H):
                u = 2 * b + h
                zabs = sbZ.tile([128, N * N], bf16, tag="zabs")
                R2 = Rbank[:, 32 * (u % 2):32 * (u % 2) + 32]
                for c in range(NCHUNK):
                    zs = zabs[:, 512 * c:512 * (c + 1)]
                    route = CH[u][c]
                    if route == "A":
                        zc = psZ.tile([128, 512], f32, tag="zc")
                        nc.tensor.matmul(
                            zc[:], eler[:, h * D:(h + 1) * D],
                            ssel[:, 512 * c:512 * (c + 1)],
                            start=True, stop=True)
                        nc.scalar.activation(zs, zc[:], Act.Relu)
                    else:
                        erT, elT32 = bc[h]
                        eng = nc.vector if route == "D" else nc.gpsimd
                        for lam in range(8):
                            eng.tensor_scalar(
                                zabs[:, 512 * c + N * lam:
                                     512 * c + N * (lam + 1)],
                                erT[:], elT32[:, 8 * c + lam:8 * c + lam + 1],
                                0.0, mybir.AluOpType.add, mybir.AluOpType.max)
                    # e-reduce: stationary zabs slice, moving 0.8*a_h col
                    for m in range(4):
                        nc.tensor.matmul(
                            R2[:, 4 * c + m:4 * c + m + 1],
                            zabs[:, 512 * c + 128 * m:512 * c + 128 * (m + 1)],
                            acol[:, h:h + 1],
                            start=True, stop=True, skip_group_check=True)

                # exp (dense), transpose, downcast, gather
                stagedE = sbU.tile([128, 32], f32, tag="stE",
                                   name=f"stE{u}", bufs=2)
                nc.scalar.activation(stagedE[:], R2[:], Act.Exp)
                pT_ps = psT.tile([32, 128], f32, tag="sm", name=f"pTp{u}")
                nc.tensor.matmul(pT_ps[:], stagedE[:], ident,
                                 is_transpose=True)
                pTu = sbU.tile([32, 128], bf16, tag="pTu",
                               name=f"pTu{u}", bufs=2)
                nc.vector.tensor_copy(pTu[:], pT_ps[:])
                dma_eng = nc.sync if u % 2 == 0 else nc.gpsimd
                for jl0 in range(2):
                    dma_eng.dma_start(
                        pT2[32 * jl0:32 * (jl0 + 1), 64 * u:64 * (u + 1)],
                        pTu[:, 64 * jl0:64 * (jl0 + 1)])

            # aggregate/normalize this batch now so the tail overlaps the
            # remaining batches' main compute.
            oT = psT.tile([D, N], f32, tag="sm", name=f"oT{b}")
            for h in range(H):
                u = 2 * b + h
                ag = psT.tile([N, D + 1], f32, tag="sm", name="ag")
                nc.tensor.matmul(
                    ag[:], pT2[:, 64 * u:64 * (u + 1)],
                    elgs[(b, h)][:], start=True, stop=True)
                r_u = sbU.tile([N, 1], f32, tag="r")
                nc.vector.reciprocal(r_u[:], ag[:, D:D + 1])
                t_h = sbU.tile([N, D], f32, tag="th", name=f"th{h}")
                nc.vector.tensor_scalar(t_h[:], ag[:, 0:D], r_u[:], None,
                                        mybir.AluOpType.mult)
                nc.tensor.matmul(oT[:], t_h[:], ident[0:N, 0:N],
                                 is_transpose=True,
                                 start=(h == 0), stop=(h == 1),
                                 skip_group_check=True)
            # y = 0.5*(t0+t1)^T + bias
            nc.scalar.activation(y_all[:, N * b:N * (b + 1)], oT[:],
                                 Act.Identity, bias=bias_ap, scale=0.5)
            y_src = bass.AP(
                tensor=y_all.tensor, offset=y_all.offset + N * b,
                ap=[[B_LOC * N, 128], [1, N]])
            y_dst = bass.AP(
                tensor=y_d.tensor, offset=y_d.offset + 128 * N * b,
                ap=[[N, 128], [1, N]])
            nc.sync.dma_start(y_dst, y_src)

    nc.compile()
    _cache["nc"] = nc
    return nc


def _pack_bf16(a):
    """[P, 2k] f32 -> [P, k] f32 bit-packed bf16 pairs (little-endian)."""
    import ml_dtypes
    ab = a.astype(ml_dtypes.bfloat16).view(np.uint16)
    return (ab[:, 0::2].astype(np.uint32)
            | (ab[:, 1::2].astype(np.uint32) << 16)).view(np.float32)


def _pack_fp8(a):
    """[P, 4k] f32 -> [P, k] f32 bit-packed fp8e4m3 quads."""
    import ml_dtypes
    ab = a.astype(ml_dtypes.float8_e4m3fn).view(np.uint8)
    return (ab[:, 0::4].astype(np.uint32)
            | (ab[:, 1::4].astype(np.uint32) << 8)
            | (ab[:, 2::4].astype(np.uint32) << 16)
            | (ab[:, 3::4].astype(np.uint32) << 24)).view(np.float32)


def _slot_of_r(r):
    """Virtual node r -> pair slot (chunk c, lane lam)."""
    jl0, c, jhi = r >> 5, (r >> 2) & 7, r & 3
    return c, 2 * jhi + jl0


def _make_blobs(x, Wl, Wr, attn_a, bias):
    """Host-side prep: per-core input blobs [128, NCOLS] float32."""
    x = np.asarray(x, np.float32)
    Wl = np.asarray(Wl, np.float32)
    Wr = np.asarray(Wr, np.float32)
    attn_a = np.asarray(attn_a, np.float32)
    bias = np.asarray(bias, np.float32)

    wl_u = np.einsum("whd,hd->wh", Wl.reshape(W, H, D), attn_a) * NEG_SLOPE
    wlr = np.concatenate([Wl, Wr, wl_u], axis=1)              # [128, 514]

    # selector: el row r -> its slot's 64 pair columns; er row 64+i -> every
    # chunk's lane columns with pair-i = i.
    s_sel = np.zeros((128, N * N), np.float32)
    for r in range(N):
        c, lam = _slot_of_r(r)
        s_sel[r, 512 * c + N * lam:512 * c + N * (lam + 1)] = 1.0
    loc = np.arange(N * N)
    s_sel[N + loc % N, loc] = 1.0

    # slot-order permutation for xbt: col k = 8c + lam holds node r(c, lam)
    tau = np.zeros(N, np.int64)
    for r in range(N):
        c, lam = _slot_of_r(r)
        tau[8 * c + lam] = r

    a2 = (1.0 - NEG_SLOPE) * attn_a                           # [H, 128]
    a_col = np.stack([a2[0], a2[1]], axis=1)                  # [128, 2]
    bias_f = 0.5 * (bias.reshape(H, D)[0] + bias.reshape(H, D)[1])
    ident = np.eye(128, dtype=np.float32)

    common = np.concatenate(
        [_pack_bf16(wlr), _pack_bf16(a_col),
         bias_f.reshape(128, 1), ident], axis=1)
    sel_packed = np.ascontiguousarray(_pack_fp8(s_sel))

    blobs = []
    for core in range(N_CORES):
        xs = x[core * B_LOC:(core + 1) * B_LOC]    # [4, 128, 64]
        xsec = xs.transpose(1, 0, 2).reshape(128, B_LOC * N)
        xsect = xs[:, :, tau].transpose(1, 0, 2).reshape(128, B_LOC * N)
        ba = np.ascontiguousarray(np.concatenate(
            [_pack_bf16(xsec), _pack_bf16(xsect), common], axis=1))
        blobs.append({"blob": ba, "blobS": sel_packed})
    return blobs


def kernel(x, Wl, Wr, attn_a, bias):
    nc = _build()
    blobs = _make_blobs(x, Wl, Wr, attn_a, bias)
    in_maps = [blobs[c] for c in range(N_CORES)]
    res = run_bass_kernel_spmd(nc, in_maps, list(range(N_CORES)))
    out = np.concatenate([res.results[c]["y"] for c in range(N_CORES)],
                         axis=0)
    return out.astype(np.float32)


# revision 19
# speedup vs baseline: 1.1504x; 1.1504x over previous
"""Trainium2 Bass kernel for DGLFeatureGAT (dense GATv2 over complete graph).

Reference computation (per batch b, head h; N=64 nodes, D=128 feat dim):
    el = xn @ Wl,  er = xn @ Wr                      # [N, H, D]
    e[h,i,j] = sum_d a[h,d] * lrelu(el[j,h,d] + er[i,h,d])
    alpha = softmax_j(e);  rst[i,h,d] = sum_j alpha[h,i,j] el[j,h,d] + bias
    out = mean_h(rst) transposed to [D, N]

Exact decomposition (slope s=0.2):
    lrelu(z) = (1-s)*relu(z) + s*z, so with z = el_j + er_i:
    e = (1-s)*sum_d a_d relu(z_d) + s*u_j + s*v_i    (u = a.el, v = a.er)
      - v_i constant over j -> dropped (softmax invariant)
      - u_j enters as g_j = exp(s*u_j) multiplied into the aggregation rhs,
        with one extra rhs column accumulating the softmax normalizer.

Pair-space layout per (b, h) "unit" (4096 pairs = 8 chunks x 512):
    pair p = 512c + 64*lam + i.  Node-j slots use the bit-rotated index
    r = 32*jl0 + 4c + jhi (lam = 2*jhi + jl0): the e-reduce then lands e
    DENSE in PSUM ([128, 32]: partition 64*jl0+i, col 4c+jhi), one exp per
    unit, PE-transpose + copy + two standard-slice DMAs give pT2[r, 64u+i].
    All j-side tensors (el, elg, g, selector rows) consistently use
    r-indexing; i-side and the output stay in natural order.

Engines:
    z+relu per chunk, route table CH:
      A = PE z-form matmul (bf16 stationary [el;er], fp8 0/1 selector
          moving) + ACT Relu from PSUM (612ns)
      D = DVE: tensor_tensor add of broadcast views (elT slot-expanded +
          erT tiled, 2x) + tensor_scalar relu (4x), SBUF bf16 (491ns)
      P = Pool tensor_scalar relu(erT + elT32_col) per slot (8x184ns)
    e-reduce: 4 matmuls/chunk, STATIONARY = zabs [128,128] slice, moving =
    0.8*a_h column (out free size 1 -> ~free on PE).
    exp: one ACT call per unit [128, 32].  Aggregation fp32 on PE.

Sharding: pure data-parallel, B=32 -> 4 batches per core x 8 cores.
"""

import numpy as np
from contextlib import ExitStack

import concourse.bass as bass
import concourse.bacc as bacc
import concourse.tile as tile
from concourse import mybir
from concourse.bass_utils import run_bass_kernel_spmd

f32 = mybir.dt.float32
bf16 = mybir.dt.bfloat16
fp8 = mybir.dt.float8e4
Act = mybir.ActivationFunctionType

B, W, F, H, D = 32, 128, 64, 2, 128
NEG_SLOPE = 0.2
N_CORES = 8
B_LOC = B // N_CORES            # 4 batches per core
N = F                           # 64 nodes
NCHUNK = 8                      # 512-col chunks of the 4096 pair space
NU = B_LOC * H                  # 8 units per core

# blobA column layout (float32 bits; bf16 regions are packed); the fp8
# selector ships separately as blobS so head compute starts immediately.
OFF_X = 0                            # [128, 128]: x[b] bf16 [w, r] natural
OFF_XT = OFF_X + B_LOC * N // 2      # [128, 128]: x[b] bf16 slot-order tau
OFF_WLR = OFF_XT + B_LOC * N // 2    # [128, 257]: Wl | Wr | wl_u bf16
OFF_ACOL = OFF_WLR + 257             # [128, 1]: 0.8*a bf16 pair (h0, h1)
OFF_BIAS = OFF_ACOL + 1              # [128, 1]: fused output bias f32
OFF_I128 = OFF_BIAS + 1              # [128, 128]: identity f32
NCOLS = OFF_I128 + 128
NCOLS_S = N * N // 4                 # selector fp8-packed

# per-chunk z route (see module docstring).  P-chunks must be spread so
# Pool stays fed in program order.
CH = {
    0: ["A", "A", "A", "A", "A", "A", "A", "A"],
    1: ["A", "A", "P", "A", "A", "A", "P", "A"],
    2: ["A", "A", "A", "P", "A", "A", "A", "P"],
    3: ["P", "D", "A", "D", "P", "D", "A", "D"],
    4: ["A", "D", "P", "D", "A", "P", "D", "D"],
    5: ["D", "A", "D", "P", "D", "A", "P", "D"],
    6: ["P", "D", "D", "A", "P", "D", "A", "D"],
    7: ["D", "P", "D", "D", "D", "P", "D", "D"],
}

_cache = {}


def _build():
    if "nc" in _cache:
        return _cache["nc"]
    nc = bacc.Bacc("TRN2", target_bir_lowering=False, debug=False)
    blob_d = nc.declare_dram_parameter("blob", [128, NCOLS], f32,
                                       isOutput=False).ap()
    blobs_d = nc.declare_dram_parameter("blobS", [128, NCOLS_S], f32,
                                        isOutput=False).ap()
    y_d = nc.declare_dram_parameter("y", [B_LOC, D, F], f32,
                                    isOutput=True).ap()

    with tile.TileContext(nc) as tc, ExitStack() as ctx:
        sb1 = ctx.enter_context(tc.tile_pool(name="sb1", bufs=1))
        sbE = ctx.enter_context(tc.tile_pool(name="sbE", bufs=2 * B_LOC))
        sbZ = ctx.enter_context(tc.tile_pool(name="sbZ", bufs=2))
        sbU = ctx.enter_context(tc.tile_pool(name="sbU", bufs=B_LOC))
        psP = ctx.enter_context(tc.tile_pool(name="psP", bufs=2, space="PSUM"))
        psT = ctx.enter_context(tc.tile_pool(name="psT", bufs=2, space="PSUM"))
        psZ = ctx.enter_context(tc.tile_pool(name="psZ", bufs=3, space="PSUM"))
        psR = ctx.enter_context(tc.tile_pool(name="psR", bufs=1, space="PSUM"))

        blob = sb1.tile([128, NCOLS], f32, tag="blob")
        nc.sync.dma_start(blob[:], blob_d)
        blobS = sb1.tile([128, NCOLS_S], f32, tag="blobS")
        nc.sync.dma_start(blobS[:], blobs_d)

        def bl(off, w):
            return blob[:, off:off + w]

        xall = bl(OFF_X, B_LOC * N // 2).bitcast(bf16)       # [128, 256]
        xtau = bl(OFF_XT, B_LOC * N // 2).bitcast(bf16)      # [128, 256]
        wlr = bl(OFF_WLR, 257).bitcast(bf16)                 # [128, 514]
        ssel = blobS[:].bitcast(fp8)                         # [128, 4096]
        acol = bl(OFF_ACOL, 1).bitcast(bf16)                 # [128, 2]
        ident = bl(OFF_I128, 128)
        bias_ap = bl(OFF_BIAS, 1)

        pT2 = sb1.tile([N, NU * N], bf16, tag="pT2")
        y_all = sb1.tile([D, B_LOC * N], f32, tag="yall")
        Rbank = psR.tile([128, 64], f32, tag="r2")   # col 32*(u%2) + 4c + m

        elgs = {}
        elers = {}
        bcs = {}
        for b in range(B_LOC):
            has_a = {h: any(r == "A" for r in CH[2 * b + h]) for h in range(H)}
            has_d = {h: any(r == "D" for r in CH[2 * b + h]) for h in range(H)}
            has_p = {h: any(r == "P" for r in CH[2 * b + h]) for h in range(H)}

            xb = xall[:, b * N:(b + 1) * N]                  # [128, 64] bf16
            xbt = xtau[:, b * N:(b + 1) * N]                 # [128, 64] bf16

            proj = psP.tile([N, 512], f32, tag="sm", name="proj")
            nc.tensor.matmul(proj[:], xb, wlr[:, 0:512],
                             start=True, stop=True)
            proju = psP.tile([N, 2], f32, tag="sm", name="proju")
            nc.tensor.matmul(proju[:], xb, wlr[:, 512:514],
                             start=True, stop=True)

            # stacked [el; er] (rows 0:64 el, 64:128 er) bf16 for the A
            # route z-form; el half doubles as the 4x elg source.
            eler = sbE.tile([128, H * D], bf16, tag="eler")
            nc.vector.tensor_copy(eler[0:N, :], proj[:, 0:H * D])
            if any(has_a.values()):
                nc.vector.tensor_copy(eler[N:128, :],
                                      proj[:, H * D:2 * H * D])

            g_b = sbU.tile([N, H], f32, tag="g")   # g[r,h] = exp(s*u)
            nc.scalar.activation(g_b[:], proju[:], Act.Exp)

            # elg[r, 0:D] = el[r, :] * g_r ; elg[r, D] = g_r   (4x DVE)
            for h in range(H):
                elg = sbU.tile([N, D + 1], bf16, tag=f"elg{b}{h}",
                               name=f"elg{b}{h}", bufs=1)
                nc.vector.tensor_scalar(
                    elg[:, 0:D], eler[0:N, h * D:(h + 1) * D],
                    g_b[:, h:h + 1], None, mybir.AluOpType.mult)
                nc.vector.tensor_copy(elg[:, D:D + 1], g_b[:, h:h + 1])
                elgs[(b, h)] = elg

            # transposed projections (slot order tau) for the SBUF routes
            bc = {}
            for h in range(H):
                if not (has_d[h] or has_p[h]):
                    continue
                prT = psP.tile([128, 128], f32, tag="sm", name=f"prT{b}{h}")
                nc.tensor.matmul(prT[:, 0:N], wlr[:, h * D:(h + 1) * D],
                                 xbt, start=True, stop=True)
                nc.tensor.matmul(prT[:, N:2 * N],
                                 wlr[:, H * D + h * D:H * D + (h + 1) * D],
                                 xb, start=True, stop=True)
                erT = sbE.tile([128, N], bf16, tag="erT", name=f"erT{b}{h}")
                nc.vector.tensor_copy(erT[:], prT[:, N:2 * N])
                elT32 = sbE.tile([128, N], f32, tag="elT32",
                                 name=f"elT32{b}{h}")
                nc.vector.tensor_copy(elT32[:], prT[:, 0:N])
                bc[h] = (erT, elT32)
            elers[b] = eler
            bcs[b] = bc

        for b in range(B_LOC):
            eler = elers[b]
            bc = bcs[b]
            for h in range(H):
                u = 2 * b + h
                zabs = sbZ.tile([128, N * N], bf16, tag="zabs")
                R2 = Rbank[:, 32 * (u % 2):32 * (u % 2) + 32]
                for c in range(NCHUNK):
                    zs = zabs[:, 512 * c:512 * (c + 1)]
                    route = CH[u][c]
                    if route == "A":
                        zc = psZ.tile([128, 512], f32, tag="zc")
                        nc.tensor.matmul(
                            zc[:], eler[:, h * D:(h + 1) * D],
                            ssel[:, 512 * c:512 * (c + 1)],
                            start=True, stop=True)
                        nc.scalar.activation(zs, zc[:], Act.Relu)
                    else:
                        erT, elT32 = bc[h]
                        eng = nc.vector if route == "D" else nc.gpsimd
                        for lam in range(8):
                            eng.tensor_scalar(
                                zabs[:, 512 * c + N * lam:
                                     512 * c + N * (lam + 1)],
                                erT[:], elT32[:, 8 * c + lam:8 * c + lam + 1],
                                0.0, mybir.AluOpType.add, mybir.AluOpType.max)
                    # e-reduce: stationary zabs slice, moving 0.8*a_h col
                    for m in range(4):
                        nc.tensor.matmul(
                            R2[:, 4 * c + m:4 * c + m + 1],
                            zabs[:, 512 * c + 128 * m:512 * c + 128 * (m + 1)],
                            acol[:, h:h + 1],
                            start=True, stop=True, skip_group_check=True)

                # exp (dense), transpose, downcast, gather
                stagedE = sbU.tile([128, 32], f32, tag="stE",
                                   name=f"stE{u}", bufs=2)
                nc.scalar.activation(stagedE[:], R2[:], Act.Exp)
                pT_ps = psT.tile([32, 128], f32, tag="sm", name=f"pTp{u}")
                nc.tensor.matmul(pT_ps[:], stagedE[:], ident,
                                 is_transpose=True)
                pTu = sbU.tile([32, 128], bf16, tag="pTu",
                               name=f"pTu{u}", bufs=2)
                nc.vector.tensor_copy(pTu[:], pT_ps[:])
                dma_eng = nc.sync
                for jl0 in range(2):
                    dma_eng.dma_start(
                        pT2[32 * jl0:32 * (jl0 + 1), 64 * u:64 * (u + 1)],
                        pTu[:, 64 * jl0:64 * (jl0 + 1)])

            # aggregate/normalize this batch now so the tail overlaps the
            # remaining batches' main compute.
            oT = psT.tile([D, N], f32, tag="sm", name=f"oT{b}")
            for h in range(H):
                u = 2 * b + h
                ag = psT.tile([N, D + 1], f32, tag="sm", name="ag")
                nc.tensor.matmul(
                    ag[:], pT2[:, 64 * u:64 * (u + 1)],
                    elgs[(b, h)][:], start=True, stop=True)
                r_u = sbU.tile([N, 1], f32, tag="r")
                nc.vector.reciprocal(r_u[:], ag[:, D:D + 1])
                t_h = sbU.tile([N, D], f32, tag="th", name=f"th{h}")
                nc.vector.tensor_scalar(t_h[:], ag[:, 0:D], r_u[:], None,
                                        mybir.AluOpType.mult)
                nc.tensor.matmul(oT[:], t_h[:], ident[0:N, 0:N],
                                 is_transpose=True,
                                 start=(h == 0), stop=(h == 1),
                                 skip_group_check=True)
            # y = 0.5*(t0+t1)^T + bias
            nc.scalar.activation(y_all[:, N * b:N * (b + 1)], oT[:],
                                 Act.Identity, bias=bias_ap, scale=0.5)
            y_src = bass.AP(
                tensor=y_all.tensor, offset=y_all.offset + N * b,
                ap=[[B_LOC * N, 128], [1, N]])
            y_dst = bass.AP(
                tensor=y_d.tensor, offset=y_d.offset + 128 * N * b,
                ap=[[N, 128], [1, N]])
            nc.sync.dma_start(y_dst, y_src)

    nc.compile()
    _cache["nc"] = nc
    return nc


def _pack_bf16(a):
    """[P, 2k] f32 -> [P, k] f32 bit-packed bf16 pairs (little-endian)."""
    import ml_dtypes
    ab = a.astype(ml_dtypes.bfloat16).view(np.uint16)
    return (ab[:, 0::2].astype(np.uint32)
            | (ab[:, 1::2].astype(np.uint32) << 16)).view(np.float32)


def _pack_fp8(a):
    """[P, 4k] f32 -> [P, k] f32 bit-packed fp8e4m3 quads."""
    import ml_dtypes
    ab = a.astype(ml_dtypes.float8_e4m3fn).view(np.uint8)
    return (ab[:, 0::4].astype(np.uint32)
            | (ab[:, 1::4].astype(np.uint32) << 8)
            | (ab[:, 2::4].astype(np.uint32) << 16)
            | (ab[:, 3::4].astype(np.uint32) << 24)).view(np.float32)


def _slot_of_r(r):
    """Virtual node r -> pair slot (chunk c, lane lam)."""
    jl0, c, jhi = r >> 5, (r >> 2) & 7, r & 3
    return c, 2 * jhi + jl0


def _make_blobs(x, Wl, Wr, attn_a, bias):
    """Host-side prep: per-core input blobs [128, NCOLS] float32."""
    x = np.asarray(x, np.float32)
    Wl = np.asarray(Wl, np.float32)
    Wr = np.asarray(Wr, np.float32)
    attn_a = np.asarray(attn_a, np.float32)
    bias = np.asarray(bias, np.float32)

    wl_u = np.einsum("whd,hd->wh", Wl.reshape(W, H, D), attn_a) * NEG_SLOPE
    wlr = np.concatenate([Wl, Wr, wl_u], axis=1)              # [128, 514]

    # selector: el row r -> its slot's 64 pair columns; er row 64+i -> every
    # chunk's lane columns with pair-i = i.
    s_sel = np.zeros((128, N * N), np.float32)
    for r in range(N):
        c, lam = _slot_of_r(r)
        s_sel[r, 512 * c + N * lam:512 * c + N * (lam + 1)] = 1.0
    loc = np.arange(N * N)
    s_sel[N + loc % N, loc] = 1.0

    # slot-order permutation for xbt: col k = 8c + lam holds node r(c, lam)
    tau = np.zeros(N, np.int64)
    for r in range(N):
        c, lam = _slot_of_r(r)
        tau[8 * c + lam] = r

    a2 = (1.0 - NEG_SLOPE) * attn_a                           # [H, 128]
    a_col = np.stack([a2[0], a2[1]], axis=1)                  # [128, 2]
    bias_f = 0.5 * (bias.reshape(H, D)[0] + bias.reshape(H, D)[1])
    ident = np.eye(128, dtype=np.float32)

    common = np.concatenate(
        [_pack_bf16(wlr), _pack_bf16(a_col),
         bias_f.reshape(128, 1), ident], axis=1)
    sel_packed = np.ascontiguousarray(_pack_fp8(s_sel))

    blobs = []
    for core in range(N_CORES):
        xs = x[core * B_LOC:(core + 1) * B_LOC]    # [4, 128, 64]
        xsec = xs.transpose(1, 0, 2).reshape(128, B_LOC * N)
        xsect = xs[:, :, tau].transpose(1, 0, 2).reshape(128, B_LOC * N)
        ba = np.ascontiguousarray(np.concatenate(
            [_pack_bf16(xsec), _pack_bf16(xsect), common], axis=1))
        blobs.append({"blob": ba, "blobS": sel_packed})
    return blobs


def kernel(x, Wl, Wr, attn_a, bias):
    nc = _build()
    blobs = _make_blobs(x, Wl, Wr, attn_a, bias)
    in_maps = [blobs[c] for c in range(N_CORES)]
    res = run_bass_kernel_spmd(nc, in_maps, list(range(N_CORES)))
    out = np.concatenate([res.results[c]["y"] for c in range(N_CORES)],
                         axis=0)
    return out.astype(np.float32)
